# revision 1
# baseline (speedup 1.0000x reference)
"""Causal self-attention (RoPE + QK-RMSNorm) Trainium2 kernel, 8-core tensor-parallel.

Problem: B=4, S=2048, E=2048, H=16 heads, D=128, fp32.
Sharding: tensor-parallel over heads -- each core computes 2 heads end-to-end
(QKV projection, RoPE, QK-norm, causal attention, output projection) and
returns a partial output [B*S, E]; the host sums the 8 partials.

Per-core data layouts (contraction dims on partitions):
  xT   [E, B*S]    input transposed on host (f32r)
  QT/KT [D, S]     per (b,h); rows host-permuted to [even dims | odd dims] so
                   RoPE pairs become partition halves (swap via one matmul
                   against a +-1 permutation matrix J)
  VT   [D, S] -> V [S, D] via PE transpose (ctx matmul needs V natural)
  scores^T [k, q]  softmax denominator via all-ones matmul (partition sums);
                   ctx^T = V.T @ exp(scores^T) -- no transposes in attention
  ctxT [D, S]      stationary operand of the output projection

Numerics: matmuls in f32r (~1.5e-4 rel err, full PE rate at N>=256); softmax
without max-subtraction (qk-norm bounds |score| <= sqrt(D): exp <= 9e4, no
overflow); reciprocal/rsqrt via ACT Exp(-a*Ln(x)) so only one ACT table set
(natural_log_exp) is ever loaded.
"""

import sys

sys.path.insert(0, "/opt/trn_rl_repo")

import numpy as np
from contextlib import ExitStack

import concourse.bass as bass
import concourse.mybir as mybir
import concourse.tile as tile
from concourse import bacc
from concourse.bass_utils import run_bass_kernel_spmd

# Keep every ACT function this kernel uses (Exp/Ln/Square/Copy) resolvable
# only via the combined natural_log_exp_and_others table set; otherwise the
# table-load inserter alternates between exp_and_others and natural_log and
# pays a ~2.7us ACT_TABLE_LOAD on nearly every activation (~500us/run).
_orig_get_act_tables = bacc.get_activation_tables
_COMBINED = "natural_log_exp_and_others"
_KEEP = {mybir.ActivationFunctionType.Exp, mybir.ActivationFunctionType.Ln,
         mybir.ActivationFunctionType.Square, mybir.ActivationFunctionType.Copy}


def _patched_get_act_tables(arch):
    tables = _orig_get_act_tables(arch)
    if _COMBINED in tables and _KEEP <= tables[_COMBINED]:
        for name, funcs in tables.items():
            if name != _COMBINED:
                tables[name] = funcs - _KEEP
    return tables


bacc.get_activation_tables = _patched_get_act_tables

F32 = mybir.dt.float32
F32R = mybir.dt.float32r
AF = mybir.ActivationFunctionType

N_CORES = 8
N_HEAD = 16
ROPE_BASE = 10000.0
QK_NORM_EPS = 1e-5

B, S, E = 4, 2048, 2048
D = E // N_HEAD          # 128
HPC = N_HEAD // N_CORES  # heads per core


def build_kernel(b_=B, s_=S, repeat=1):
    """Build the per-core Bass program for batch size b_ and seqlen s_."""
    ROWS = b_ * s_
    QB = min(512, s_)     # q-block width in attention
    NQB = s_ // QB
    KPQ = QB // 128       # k-tiles spanned by one q-block (diag band width)
    NE = E // 128
    RC = min(512, s_)     # phase-A row chunk

    nc = bacc.Bacc("TRN2", target_bir_lowering=False, debug=False)

    xT = nc.dram_tensor("xT", [E, ROWS], F32R, kind="ExternalInput").ap()
    wq = nc.dram_tensor("wq", [E, HPC * D], F32R, kind="ExternalInput").ap()
    wk = nc.dram_tensor("wk", [E, HPC * D], F32R, kind="ExternalInput").ap()
    wv = nc.dram_tensor("wv", [E, HPC * D], F32R, kind="ExternalInput").ap()
    wp = nc.dram_tensor("wp", [HPC * D, E], F32R, kind="ExternalInput").ap()
    cos2 = nc.dram_tensor("cos2", [128, s_], F32, kind="ExternalInput").ap()
    sin2 = nc.dram_tensor("sin2", [128, s_], F32, kind="ExternalInput").ap()
    jmat = nc.dram_tensor("jmat", [128, 128], F32R, kind="ExternalInput").ap()
    trimask = nc.dram_tensor("trimask", [128, 128], F32R, kind="ExternalInput").ap()
    ident = nc.dram_tensor("ident", [128, 128], F32R, kind="ExternalInput").ap()
    onesd = nc.dram_tensor("onesd", [128, 128], F32R, kind="ExternalInput").ap()
    zerod = nc.dram_tensor("zerod", [128, 128], F32R, kind="ExternalInput").ap()
    out = nc.dram_tensor("out", [ROWS, E], F32, kind="ExternalOutput").ap()

    LN_SCALE = 1.0 / D
    LN_BIAS = QK_NORM_EPS
    EXP_SCALE = -0.5
    EXP_BIAS = -0.25 * float(np.log(D))  # folds 1/sqrt(D) into the q,k scales

    with tile.TileContext(nc) as tc, ExitStack() as ctx:
        wpool = ctx.enter_context(tc.tile_pool(name="weights", bufs=1))
        const = ctx.enter_context(tc.tile_pool(name="const", bufs=1))
        xtp = ctx.enter_context(tc.tile_pool(name="xt", bufs=2))
        qkv = ctx.enter_context(tc.tile_pool(name="qkv", bufs=1))
        tmp = ctx.enter_context(tc.tile_pool(name="tmp", bufs=2))
        expp = ctx.enter_context(tc.tile_pool(name="expp", bufs=2))
        ctxp = ctx.enter_context(tc.tile_pool(name="ctxp", bufs=1))
        outp = ctx.enter_context(tc.tile_pool(name="outp", bufs=2))

        # resident weights: [128, NE, HPC*D] with contraction slice e on free dim
        wq_s = wpool.tile([128, NE, HPC * D], F32R, tag="wqs")
        wk_s = wpool.tile([128, NE, HPC * D], F32R, tag="wks")
        wv_s = wpool.tile([128, NE, HPC * D], F32R, tag="wvs")
        wp_s = wpool.tile([128, HPC, E], F32R, tag="wps")
        nc.sync.dma_start(out=wq_s, in_=wq.rearrange("(ne p) m -> p ne m", p=128))
        nc.sync.dma_start(out=wk_s, in_=wk.rearrange("(ne p) m -> p ne m", p=128))
        nc.sync.dma_start(out=wv_s, in_=wv.rearrange("(ne p) m -> p ne m", p=128))
        nc.sync.dma_start(out=wp_s, in_=wp.rearrange("(h p) m -> p h m", p=128))

        cos_s = const.tile([128, s_], F32, tag="cos")
        sin_s = const.tile([128, s_], F32, tag="sin")
        j_s = const.tile([128, 128], F32R, tag="jmat")
        tri_s = const.tile([128, 128], F32R, tag="tri")
        id_s = const.tile([128, 128], F32R, tag="ident")
        ones_s = const.tile([128, 128], F32R, tag="ones")
        zero_s = const.tile([128, 128], F32R, tag="zeros")
        nc.sync.dma_start(out=ones_s, in_=onesd)
        nc.sync.dma_start(out=zero_s, in_=zerod)
        bias_ln = const.tile([128, 1], F32, tag="bias_ln")
        bias_ex = const.tile([128, 1], F32, tag="bias_ex")
        nc.vector.memset(bias_ln, LN_BIAS)
        nc.vector.memset(bias_ex, EXP_BIAS)

        nc.sync.dma_start(out=cos_s, in_=cos2)
        nc.sync.dma_start(out=sin_s, in_=sin2)
        nc.sync.dma_start(out=j_s, in_=jmat)
        nc.sync.dma_start(out=tri_s, in_=trimask)
        nc.sync.dma_start(out=id_s, in_=ident)

        rep_ctx = tc.For_i(0, repeat, 1) if repeat > 1 else None
        if rep_ctx is not None:
            ctx.enter_context(rep_ctx)

        for b in range(b_):
            # ---------- phase A: QKV projection + V transpose + rope + norm
            qtn = [qkv.tile([128, s_], F32R, tag=f"qtn{h}", name=f"qtn{h}") for h in range(HPC)]
            ktn = [qkv.tile([128, s_], F32R, tag=f"ktn{h}", name=f"ktn{h}") for h in range(HPC)]
            vsb = [qkv.tile([128, s_ // 128, D], F32R, tag=f"vsb{h}", name=f"vsb{h}")
                   for h in range(HPC)]

            with tc.tile_pool(name=f"psA{b}", bufs=1, space="PSUM") as psA:
                for rc in range(s_ // RC):
                    col0 = b * s_ + rc * RC
                    csl = slice(rc * RC, rc * RC + RC)
                    p_q = [psA.tile([128, RC], F32, tag=f"pq{h}", name=f"pq{h}") for h in range(HPC)]
                    p_k = [psA.tile([128, RC], F32, tag=f"pk{h}", name=f"pk{h}") for h in range(HPC)]
                    p_vt = [psA.tile([128, RC], F32, tag=f"pvt{h}", name=f"pvt{h}") for h in range(HPC)]

                    for e in range(NE):
                        xt = xtp.tile([128, RC], F32R, tag="xt")
                        nc.sync.dma_start(
                            out=xt, in_=xT[e * 128:(e + 1) * 128, col0:col0 + RC])
                        st, sp = (e == 0), (e == NE - 1)
                        for h in range(HPC):
                            hsl = slice(h * D, (h + 1) * D)
                            nc.tensor.matmul(p_q[h], wq_s[:, e, hsl], xt,
                                             start=st, stop=sp)
                            nc.tensor.matmul(p_k[h], wk_s[:, e, hsl], xt,
                                             start=st, stop=sp)
                            nc.tensor.matmul(p_vt[h], wv_s[:, e, hsl], xt,
                                             start=st, stop=sp)

                    # psum-freeing ops first (DVE copies + ACT squares),
                    # rope tails queue behind on DVE/ACT/gpsimd
                    raws, sqs = [], []
                    for h in range(HPC):
                        for psrc in (p_q[h], p_k[h]):
                            raw = tmp.tile([128, RC], F32R, tag="raw",
                                           name="raw", bufs=4)
                            nc.vector.tensor_copy(raw, psrc)
                            sq = tmp.tile([128, RC], F32R, tag="sq",
                                          name="sq", bufs=4)
                            nc.scalar.activation(sq, psrc, AF.Square)
                            raws.append(raw)
                            sqs.append(sq)
                    vt_sbs = []
                    for h in range(HPC):
                        vt_sb = tmp.tile([128, RC], F32R, tag="vt", name="vt",
                                         bufs=2)
                        nc.vector.tensor_copy(vt_sb, p_vt[h])
                        vt_sbs.append(vt_sb)

                    for h in range(HPC):
                        for which, dst in ((0, qtn[h]), (1, ktn[h])):
                            raw = raws[2 * h + which]
                            sq = sqs[2 * h + which]
                            p_ss = psA.tile([128, RC], F32, tag="scratch")
                            nc.tensor.matmul(p_ss, ones_s, sq, start=True, stop=True)
                            lnt = tmp.tile([128, RC], F32, tag="t1", name="lnt",
                                           bufs=3)
                            nc.scalar.activation(lnt, p_ss, AF.Ln,
                                                 scale=LN_SCALE, bias=bias_ln)
                            rq = tmp.tile([128, RC], F32, tag="sq", name="rq",
                                          bufs=4)
                            nc.scalar.activation(rq, lnt, AF.Exp,
                                                 scale=EXP_SCALE, bias=bias_ex)
                            p_jq = psA.tile([128, RC], F32, tag="scratch")
                            nc.tensor.matmul(p_jq, j_s, raw, start=True, stop=True)
                            t1 = tmp.tile([128, RC], F32, tag="t1", name="t1",
                                          bufs=3)
                            nc.gpsimd.tensor_mul(t1, raw, cos_s[:, csl])
                            t2 = tmp.tile([128, RC], F32, tag="t2", name="t2",
                                          bufs=3)
                            nc.vector.tensor_mul(t2, p_jq, sin_s[:, csl])
                            t3 = tmp.tile([128, RC], F32, tag="t2", name="t3",
                                          bufs=3)
                            nc.gpsimd.tensor_add(t3, t1, t2)
                            nc.gpsimd.tensor_mul(dst[:, csl], t3, rq)

                    # V transposes after psum-freeing work is queued
                    for h in range(HPC):
                        for pt in range(RC // 128):
                            kt = (rc * RC) // 128 + pt
                            p_tr = psA.tile([128, 128], F32R, tag="scratch")
                            nc.tensor.transpose(
                                p_tr, vt_sbs[h][:, pt * 128:(pt + 1) * 128], id_s)
                            nc.vector.tensor_copy(vsb[h][:, kt, :], p_tr)

            # ---------- phase B+C: attention with interleaved projection --
            ctxTs = [ctxp.tile([128, s_], F32R, tag=f"ctxT{h}", name=f"ctxT{h}") for h in range(HPC)]
            with tc.tile_pool(name=f"psB{b}", bufs=1, space="PSUM") as psB:
                for qb in range(NQB):
                    for h in range(HPC):
                        qsl = slice(qb * QB, (qb + 1) * QB)
                        p_ctx = psB.tile([128, QB], F32, tag="p_ctx", bufs=1)
                        p_rs = psB.tile([128, QB], F32, tag="p_rs", bufs=1)
                        n_kt = (qb + 1) * KPQ
                        for g in range(max(1, n_kt // 2)):
                            kts = [kt for kt in (2 * g, 2 * g + 1) if kt < n_kt]
                            p_s = psB.tile([128, 2 * QB], F32, tag="p_s", bufs=2)
                            for i, kt in enumerate(kts):
                                nc.tensor.matmul(
                                    p_s[:, i * QB:(i + 1) * QB],
                                    ktn[h][:, kt * 128:(kt + 1) * 128],
                                    qtn[h][:, qsl], start=True, stop=True)
                            ex = expp.tile([128, 2 * QB], F32R, tag="ex")
                            rels = [kt - qb * KPQ for kt in kts]
                            if all(r < 0 for r in rels):
                                nc.scalar.activation(ex[:, :len(kts) * QB],
                                                     p_s[:, :len(kts) * QB], AF.Exp)
                            else:
                                for i, kt in enumerate(kts):
                                    rel = rels[i]
                                    esl = ex[:, i * QB:(i + 1) * QB]
                                    psl = p_s[:, i * QB:(i + 1) * QB]
                                    if rel < 0:
                                        nc.scalar.activation(esl, psl, AF.Exp)
                                        continue
                                    for z in range(rel):
                                        nc.vector.tensor_copy(
                                            esl[:, z * 128:(z + 1) * 128], zero_s)
                                    nc.scalar.activation(
                                        esl[:, rel * 128:], psl[:, rel * 128:], AF.Exp)
                                    nc.vector.tensor_mul(
                                        esl[:, rel * 128:(rel + 1) * 128],
                                        esl[:, rel * 128:(rel + 1) * 128], tri_s)
                            for i, kt in enumerate(kts):
                                nc.tensor.matmul(p_ctx, vsb[h][:, kt, :],
                                                 ex[:, i * QB:(i + 1) * QB],
                                                 start=(kt == 0), stop=(kt == n_kt - 1))
                            for i, kt in enumerate(kts):
                                nc.tensor.matmul(p_rs, ones_s,
                                                 ex[:, i * QB:(i + 1) * QB],
                                                 start=(kt == 0), stop=(kt == n_kt - 1))
                        # 1/rowsum via Exp(-Ln(x)), broadcast over partitions
                        lnr = tmp.tile([128, QB], F32, tag="t1", name="lnr", bufs=3)
                        nc.scalar.activation(lnr, p_rs, AF.Ln)
                        rs = tmp.tile([128, QB], F32, tag="t2", name="rs", bufs=3)
                        nc.scalar.activation(rs, lnr, AF.Exp, scale=-1.0)
                        nc.vector.tensor_mul(ctxTs[h][:, qsl], p_ctx, rs)

                    # output projection for this q-block's row tiles; the
                    # out-DMA overlaps the next q-block's attention compute
                    for rt in range(qb * QB // 128, (qb + 1) * QB // 128):
                        rsl = slice(rt * 128, (rt + 1) * 128)
                        o_sb = outp.tile([128, E], F32, tag="o_sb")
                        for half in range(2):
                            p_o = psB.tile([128, E // 2], F32, tag="po", bufs=1)
                            for h in range(HPC):
                                for nch in range(2):
                                    off = half * (E // 2)
                                    nc.tensor.matmul(
                                        p_o[:, nch * 512:(nch + 1) * 512],
                                        ctxTs[h][:, rsl],
                                        wp_s[:, h, off + nch * 512:off + (nch + 1) * 512],
                                        start=(h == 0), stop=(h == HPC - 1))
                            hsl2 = slice(half * (E // 2), (half + 1) * (E // 2))
                            eng = nc.vector.tensor_copy if half == 0 else (
                                lambda o, i: nc.scalar.activation(o, i, AF.Copy))
                            eng(o_sb[:, hsl2], p_o)
                            nc.sync.dma_start(
                                out=out[b * s_ + rt * 128: b * s_ + (rt + 1) * 128,
                                        hsl2],
                                in_=o_sb[:, hsl2])

    nc.compile()
    return nc


def host_inputs(x, w_qkv, w_proj, core, s_=None):
    """Per-core input map (numpy, all f32)."""
    b_, s_x, e = x.shape
    s_ = s_x if s_ is None else s_
    xT = np.ascontiguousarray(x.reshape(b_ * s_, e).T)

    hs = [core * HPC + i for i in range(HPC)]
    perm = np.concatenate([np.arange(0, D, 2), np.arange(1, D, 2)])
    wq_c = np.concatenate(
        [w_qkv[:, 0 * e + h * D:0 * e + (h + 1) * D][:, perm] for h in hs], axis=1)
    wk_c = np.concatenate(
        [w_qkv[:, 1 * e + h * D:1 * e + (h + 1) * D][:, perm] for h in hs], axis=1)
    wv_c = np.concatenate(
        [w_qkv[:, 2 * e + h * D:2 * e + (h + 1) * D] for h in hs], axis=1)
    wp_c = np.concatenate([w_proj[h * D:(h + 1) * D, :] for h in hs], axis=0)

    inv_freq = 1.0 / (ROPE_BASE ** (np.arange(0, D, 2, dtype=np.float64) / D))
    t = np.arange(s_, dtype=np.float64)
    freqs = np.outer(inv_freq, t)            # [64, S]
    cosT = np.cos(freqs).astype(np.float32)
    sinT = np.sin(freqs).astype(np.float32)
    cos2 = np.vstack([cosT, cosT])
    sin2 = np.vstack([sinT, sinT])

    J = np.zeros((128, 128), np.float32)
    for r in range(64):
        J[r, r + 64] = -1.0
        J[r + 64, r] = 1.0
    jmat = np.ascontiguousarray(J.T)

    ki, qi = np.meshgrid(np.arange(128), np.arange(128), indexing="ij")
    trimask = (ki <= qi).astype(np.float32)
    ident = np.eye(128, dtype=np.float32)

    return {
        "xT": xT, "wq": np.ascontiguousarray(wq_c),
        "wk": np.ascontiguousarray(wk_c), "wv": np.ascontiguousarray(wv_c),
        "wp": np.ascontiguousarray(wp_c), "cos2": cos2, "sin2": sin2,
        "jmat": jmat, "trimask": trimask, "ident": ident,
        "onesd": np.ones((128, 128), np.float32),
        "zerod": np.zeros((128, 128), np.float32),
    }


_CACHE = {}


def _get_nc(b_, s_):
    key = (b_, s_)
    if key not in _CACHE:
        _CACHE[key] = build_kernel(b_, s_)
    return _CACHE[key]


def kernel(x, w_qkv, w_proj):
    x = np.asarray(x, dtype=np.float32)
    w_qkv = np.asarray(w_qkv, dtype=np.float32)
    w_proj = np.asarray(w_proj, dtype=np.float32)
    b_, s_, e = x.shape

    nc = _get_nc(b_, s_)
    in_maps = [host_inputs(x, w_qkv, w_proj, c) for c in range(N_CORES)]
    res = run_bass_kernel_spmd(nc, in_maps, list(range(N_CORES)))
    acc = res.results[0]["out"].astype(np.float32).copy()
    for c in range(1, N_CORES):
        acc += res.results[c]["out"]
    return acc.reshape(b_, s_, e)



# revision 2
# speedup vs baseline: 1.9261x; 1.9261x over previous
"""Causal self-attention (RoPE + QK-RMSNorm) Trainium2 kernel, 8-core tensor-parallel.

v3: bf16 data movement, pipelined single-bank PSUM rounds, J-free RoPE,
consolidated softmax rowsums, causal diagonal trimming.

Problem: B=4, S=2048, E=2048, H=16 heads, D=128, fp32 in/out.
Sharding: tensor-parallel over heads -- each core computes 2 heads end-to-end
and returns a bf16 partial output [B*S, E]; the host sums the 8 partials in f32.

Per-core layouts (contraction dims on partitions):
  xT   [E, B*S] bf16   input transposed on host
  wq/wk/wv [E, 256] bf16 (q,k head-dim pre-permuted to [even|odd] for RoPE)
  wp   [256, E] f32 (f32r in kernel -- proj path kept full precision)
  QT/KT [D, S] f32r per head; V^T -> V via PE transpose (bf16)
  scores^T [k, q] f32 PSUM; ex bf16; ctx^T = V^T @ ex (f32 PSUM)
  rowsum via ones-matmul; 1/x via DVE reciprocal_approx_fast

Phases per batch:
  A: one 2MB slab DMA per 512-row chunk; 6 single-bank PSUM accumulation
     rounds (q/k/v x 2 heads) of 16 matmuls each, pipelined (bufs=3) so the
     PE never waits on the RoPE/norm chains (spread over DVE/Pool/ACT).
  B: attention per (q-block, head) with double-buffered score PSUM and
     diagonal-band trimming; softmax rowsums accumulate on Pool/DVE with one
     ones-matmul per q-block; the output projection runs one q-block behind
     attention and the last q-block's projection overlaps the next batch's
     phase A (its PSUM pool is disjoint); out DMA on the SP queue, bulk
     constants on the ACT queue.
"""

import sys

sys.path.insert(0, "/opt/trn_rl_repo")

import numpy as np
from contextlib import ExitStack

import concourse.bass as bass
import concourse.mybir as mybir
import concourse.tile as tile
from concourse import bacc
from concourse.bass_utils import run_bass_kernel_spmd

# Keep every ACT function this kernel uses (Exp/Ln/Copy) resolvable only via
# the combined natural_log_exp_and_others table set so the table-load inserter
# never alternates sets (ACT_TABLE_LOAD is ~2.7us).
_orig_get_act_tables = bacc.get_activation_tables
_COMBINED = "natural_log_exp_and_others"
_KEEP = {mybir.ActivationFunctionType.Exp, mybir.ActivationFunctionType.Ln,
         mybir.ActivationFunctionType.Square, mybir.ActivationFunctionType.Copy}


def _patched_get_act_tables(arch):
    tables = _orig_get_act_tables(arch)
    if _COMBINED in tables and _KEEP <= tables[_COMBINED]:
        for name, funcs in tables.items():
            if name != _COMBINED:
                tables[name] = funcs - _KEEP
    return tables


bacc.get_activation_tables = _patched_get_act_tables

F32 = mybir.dt.float32
F32R = mybir.dt.float32r
BF16 = mybir.dt.bfloat16
AF = mybir.ActivationFunctionType

N_CORES = 8
N_HEAD = 16
ROPE_BASE = 10000.0
QK_NORM_EPS = 1e-5

B, S, E = 4, 2048, 2048
D = E // N_HEAD          # 128
HPC = N_HEAD // N_CORES  # heads per core


def build_kernel(b_=B, s_=S, repeat=1):
    ROWS = b_ * s_
    QB = min(512, s_)     # q-block width in attention
    NQB = s_ // QB
    KPQ = QB // 128       # k-tiles spanned by one q-block
    NE = E // 128
    RC = min(512, s_)     # phase-A row chunk
    NKT = s_ // 128

    nc = bacc.Bacc("TRN2", target_bir_lowering=False, debug=False)

    xT = nc.dram_tensor("xT", [E, ROWS], BF16, kind="ExternalInput").ap()
    wq = nc.dram_tensor("wq", [E, HPC * D], BF16, kind="ExternalInput").ap()
    wk = nc.dram_tensor("wk", [E, HPC * D], BF16, kind="ExternalInput").ap()
    wv = nc.dram_tensor("wv", [E, HPC * D], BF16, kind="ExternalInput").ap()
    wp = nc.dram_tensor("wp", [HPC * D, E], F32R, kind="ExternalInput").ap()
    cos2 = nc.dram_tensor("cos2", [128, s_], F32, kind="ExternalInput").ap()
    sin2n = nc.dram_tensor("sin2n", [128, s_], F32, kind="ExternalInput").ap()
    onesf = nc.dram_tensor("onesf", [128, 128], F32R, kind="ExternalInput").ap()
    trimask = nc.dram_tensor("trimask", [128, 128], BF16, kind="ExternalInput").ap()
    ident = nc.dram_tensor("ident", [128, 128], BF16, kind="ExternalInput").ap()
    onesd = nc.dram_tensor("onesd", [128, 128], BF16, kind="ExternalInput").ap()
    out = nc.dram_tensor("out", [ROWS, E], BF16, kind="ExternalOutput").ap()

    LN_SCALE = 1.0 / D
    LN_BIAS = QK_NORM_EPS
    EXP_SCALE = -0.5
    EXP_BIAS = -0.25 * float(np.log(D))  # folds 1/sqrt(D) into the q,k scales

    with tile.TileContext(nc) as tc, ExitStack() as ctx:
        wpool = ctx.enter_context(tc.tile_pool(name="weights", bufs=1))
        const = ctx.enter_context(tc.tile_pool(name="const", bufs=1))
        xtp = ctx.enter_context(tc.tile_pool(name="xt", bufs=2))
        qkv = ctx.enter_context(tc.tile_pool(name="qkv", bufs=1))
        tmp = ctx.enter_context(tc.tile_pool(name="tmp", bufs=2))
        expp = ctx.enter_context(tc.tile_pool(name="expp", bufs=2))
        ctxp = ctx.enter_context(tc.tile_pool(name="ctxp", bufs=1))
        outp = ctx.enter_context(tc.tile_pool(name="outp", bufs=2))

        # resident weights: [128, NE, HPC*D] with contraction slice e on free dim
        wq_s = wpool.tile([128, NE, HPC * D], BF16, tag="wqs")
        wk_s = wpool.tile([128, NE, HPC * D], BF16, tag="wks")
        wv_s = wpool.tile([128, NE, HPC * D], BF16, tag="wvs")
        wp_s = wpool.tile([128, HPC, E], F32R, tag="wps")
        nc.sync.dma_start(out=wq_s, in_=wq.rearrange("(ne p) m -> p ne m", p=128))

        def issue_slab(b, rc):
            col0 = b * s_ + rc * RC
            xs = xtp.tile([128, NE, RC], BF16, tag="xs")
            nc.sync.dma_start(
                out=xs,
                in_=xT[:, col0:col0 + RC].rearrange("(ne p) m -> p ne m", p=128))
            return xs

        # slab0 goes on the SP queue right behind wq so the first QKV round
        # starts as early as possible; wk/wv stream behind it.
        slab0 = issue_slab(0, 0)
        nc.sync.dma_start(out=wk_s, in_=wk.rearrange("(ne p) m -> p ne m", p=128))
        nc.sync.dma_start(out=wv_s, in_=wv.rearrange("(ne p) m -> p ne m", p=128))

        cos_s = const.tile([128, s_], F32, tag="cos")
        sinneg_s = const.tile([128, s_], F32, tag="sinneg")
        onesf_s = const.tile([128, 128], F32R, tag="onesf")
        tri_s = const.tile([128, 128], BF16, tag="tri")
        id_s = const.tile([128, 128], BF16, tag="ident")
        ones_s = const.tile([128, 128], BF16, tag="ones")
        bias_ln = const.tile([128, 1], F32, tag="bias_ln")
        bias_ex = const.tile([128, 1], F32, tag="bias_ex")
        nc.vector.memset(bias_ln, LN_BIAS)
        nc.vector.memset(bias_ex, EXP_BIAS)

        nc.sync.dma_start(out=onesf_s, in_=onesf)
        nc.sync.dma_start(out=tri_s, in_=trimask)
        nc.sync.dma_start(out=id_s, in_=ident)
        nc.sync.dma_start(out=ones_s, in_=onesd)
        # big constants + wp go on the ACT hwdge queue, off the critical
        # SP queue (weights + x slabs)
        nc.scalar.dma_start(out=cos_s, in_=cos2)
        nc.scalar.dma_start(out=sinneg_s, in_=sin2n)
        nc.scalar.dma_start(out=wp_s, in_=wp.rearrange("(h p) m -> p h m", p=128))

        rep_ctx = tc.For_i(0, repeat, 1) if repeat > 1 else None
        if rep_ctx is not None:
            ctx.enter_context(rep_ctx)

        # projection PSUM lives in its own 2-bank pool that stays open across
        # batches, so the final q-block's projection overlaps the next
        # batch's phase A (whose pool needs only 6 banks).
        poP = ctx.enter_context(tc.tile_pool(name="poP", bufs=1, space="PSUM"))

        for b in range(b_):
            # ---------- phase A: QKV projection + RoPE + QK-norm + V^T ----
            qtn = [qkv.tile([128, s_], F32R, tag=f"qtn{h}", name=f"qtn{h}")
                   for h in range(HPC)]
            ktn = [qkv.tile([128, s_], F32R, tag=f"ktn{h}", name=f"ktn{h}")
                   for h in range(HPC)]
            vsb = [qkv.tile([128, NKT, D], BF16, tag=f"vsb{h}", name=f"vsb{h}")
                   for h in range(HPC)]

            with tc.tile_pool(name=f"psA{b}", bufs=1, space="PSUM") as psA:
                for rc in range(s_ // RC):
                    csl = slice(rc * RC, rc * RC + RC)
                    xs = slab0 if (b == 0 and rc == 0) else issue_slab(b, rc)

                    for h in range(HPC):
                        hsl = slice(h * D, (h + 1) * D)
                        for wt, kind in ((wq_s, "q"), (wk_s, "k"), (wv_s, "v")):
                            acc = psA.tile([128, RC], F32, tag="acc",
                                           name=f"acc{h}{kind}", bufs=3)
                            for e in range(NE):
                                nc.tensor.matmul(acc, wt[:, e, hsl], xs[:, e, :],
                                                 start=(e == 0), stop=(e == NE - 1))
                            if kind == "v":
                                vt_sb = tmp.tile([128, RC], BF16, tag="vt",
                                                 name="vt", bufs=2)
                                nc.vector.tensor_copy(vt_sb, acc)
                                for pt in range(RC // 128):
                                    kt = (rc * RC) // 128 + pt
                                    p_tr = psA.tile([128, 128], BF16,
                                                    tag="ptr", bufs=1)
                                    nc.tensor.transpose(
                                        p_tr, vt_sb[:, pt * 128:(pt + 1) * 128],
                                        id_s)
                                    nc.scalar.activation(vsb[h][:, kt, :], p_tr, AF.Copy)
                                continue
                            dst = qtn[h] if kind == "q" else ktn[h]
                            raw = tmp.tile([128, RC], F32R, tag="raw",
                                           name="raw", bufs=3)
                            nc.vector.tensor_copy(raw, acc)          # DVE
                            sq = tmp.tile([128, RC], BF16, tag="sq",
                                          name="sq", bufs=3)
                            nc.scalar.activation(sq, acc, AF.Square)  # ACT
                            p_ss = psA.tile([128, RC], F32, tag="pss", bufs=1)
                            nc.tensor.matmul(p_ss, ones_s, sq,
                                             start=True, stop=True)
                            lnt = tmp.tile([128, RC], F32, tag="lnt",
                                           name="lnt", bufs=2)
                            nc.scalar.activation(lnt, p_ss, AF.Ln,
                                                 scale=LN_SCALE, bias=bias_ln)
                            rq = tmp.tile([128, RC], F32, tag="rq",
                                          name="rq", bufs=2)
                            nc.scalar.activation(rq, lnt, AF.Exp,
                                                 scale=EXP_SCALE, bias=bias_ex)
                            t1 = tmp.tile([128, RC], F32, tag="t1",
                                          name="t1", bufs=2)
                            nc.gpsimd.tensor_mul(t1, raw, cos_s[:, csl])  # Pool
                            # J-free RoPE partner swap: t2[p] = -raw[p+64]*sin
                            # (p<64) / +raw[p-64]*sin (p>=64) via partition-
                            # offset reads of the [-sin; +sin] table
                            t2 = tmp.tile([128, RC], F32, tag="t2",
                                          name="t2", bufs=2)
                            nc.vector.tensor_mul(
                                t2[0:64, :], raw[64:128, :],
                                sinneg_s[64:128, csl])                 # DVE
                            nc.vector.tensor_mul(
                                t2[64:128, :], raw[0:64, :],
                                sinneg_s[0:64, csl])                   # DVE
                            t3 = tmp.tile([128, RC], F32, tag="t3",
                                          name="t3", bufs=2)
                            nc.gpsimd.tensor_add(t3, t1, t2)          # Pool
                            nc.vector.tensor_mul(dst[:, csl], t3, rq)  # DVE

            # ---------- phase B: attention + interleaved output projection --
            ctxTs = [ctxp.tile([128, s_], F32R, tag=f"ctxT{h}", name=f"ctxT{h}")
                     for h in range(HPC)]

            def emit_proj(qb):
                # output projection for q-block qb's row tiles; runs one
                # q-block behind attention so ctxT writes are long settled
                for rt in range(qb * QB // 128, (qb + 1) * QB // 128):
                    rsl = slice(rt * 128, (rt + 1) * 128)
                    o_sb = outp.tile([128, E], BF16, tag="o_sb")
                    for oc in range(4):
                        p_o = poP.tile([128, 512], F32, tag="po", bufs=2)
                        for h in range(HPC):
                            nc.tensor.matmul(
                                p_o, ctxTs[h][:, rsl],
                                wp_s[:, h, oc * 512:(oc + 1) * 512],
                                start=(h == 0), stop=(h == HPC - 1))
                        osl = o_sb[:, oc * 512:(oc + 1) * 512]
                        if oc % 2 == 0:
                            nc.vector.tensor_copy(osl, p_o)
                        else:
                            nc.scalar.activation(osl, p_o, AF.Copy)
                    nc.sync.dma_start(
                        out=out[b * s_ + rt * 128: b * s_ + (rt + 1) * 128, :],
                        in_=o_sb)

            with tc.tile_pool(name=f"psB{b}", bufs=1, space="PSUM") as psB:
                for qb in range(NQB):
                    qsl = slice(qb * QB, (qb + 1) * QB)
                    for h in range(HPC):
                        p_ctx = psB.tile([128, QB], F32, tag="p_ctx", bufs=1)
                        n_kt = (qb + 1) * KPQ
                        # rowsum: accumulate exp tiles on Pool/DVE (two
                        # independent chains), one ones-matmul at the end
                        exsA = tmp.tile([128, QB], F32R, tag="exsA",
                                        name="exsA", bufs=2)
                        exsB = tmp.tile([128, QB], F32R, tag="exsB",
                                        name="exsB", bufs=2)
                        for g in range(n_kt // 2):
                            kts = (2 * g, 2 * g + 1)
                            p_s = psB.tile([128, 2, QB], F32, tag="p_s", bufs=2)
                            rels = [kt - qb * KPQ for kt in kts]
                            for i, kt in enumerate(kts):
                                lo = max(0, rels[i]) * 128
                                if lo >= QB - 128:
                                    lo = 0  # f32r moving <256 is slower: keep full
                                nc.tensor.matmul(
                                    p_s[:, i, lo:],
                                    ktn[h][:, kt * 128:(kt + 1) * 128],
                                    qtn[h][:, qb * QB + lo:(qb + 1) * QB],
                                    start=True, stop=True,
                                    skip_group_check=(lo > 0))
                            ex = expp.tile([128, 2, QB], BF16, tag="ex")
                            if all(r < 0 for r in rels):
                                nc.scalar.activation(ex, p_s, AF.Exp)
                            else:
                                for i, kt in enumerate(kts):
                                    rel = rels[i]
                                    esl = ex[:, i, :]
                                    psl = p_s[:, i, :]
                                    if rel < 0:
                                        nc.scalar.activation(esl, psl, AF.Exp)
                                        continue
                                    if rel > 0:
                                        nc.vector.memset(esl[:, :rel * 128], 0.0)
                                    nc.scalar.activation(
                                        esl[:, rel * 128:], psl[:, rel * 128:],
                                        AF.Exp)
                                    nc.vector.tensor_mul(
                                        esl[:, rel * 128:(rel + 1) * 128],
                                        esl[:, rel * 128:(rel + 1) * 128],
                                        tri_s)
                            for i, kt in enumerate(kts):
                                lo = max(0, rels[i]) * 128
                                nc.tensor.matmul(p_ctx[:, lo:],
                                                 vsb[h][:, kt, :],
                                                 ex[:, i, lo:],
                                                 start=(kt == 0),
                                                 stop=(kt == n_kt - 1),
                                                 skip_group_check=True)
                            lo0 = max(0, rels[0]) * 128
                            lo1 = max(0, rels[1]) * 128
                            if g == 0:
                                nc.gpsimd.tensor_copy(exsA, ex[:, 0, :])
                                nc.vector.tensor_copy(exsB, ex[:, 1, :])
                            else:
                                nc.gpsimd.tensor_add(
                                    exsA[:, lo0:], exsA[:, lo0:], ex[:, 0, lo0:])
                                nc.vector.tensor_add(
                                    exsB[:, lo1:], exsB[:, lo1:], ex[:, 1, lo1:])
                        nc.gpsimd.tensor_add(exsA, exsA, exsB)
                        p_rs = psB.tile([128, QB], F32, tag="p_rs", bufs=1)
                        nc.tensor.matmul(p_rs, onesf_s, exsA,
                                         start=True, stop=True)
                        rs_inv = tmp.tile([128, QB], F32, tag="rsinv",
                                          name="rsinv", bufs=2)
                        nc.vector.reciprocal_approx_fast(rs_inv, p_rs)
                        nc.vector.tensor_mul(ctxTs[h][:, qsl], p_ctx, rs_inv)

                    if qb > 0:
                        emit_proj(qb - 1)
            # last q-block's projection runs after psB closes, so its PE work
            # overlaps the next batch's phase A (psA + poP fit in 8 banks)
            emit_proj(NQB - 1)

    nc.compile()
    return nc


def host_inputs(x, w_qkv, w_proj, core, s_=None):
    """Per-core input map (numpy)."""
    import ml_dtypes
    bf16 = ml_dtypes.bfloat16
    b_, s_x, e = x.shape
    s_ = s_x if s_ is None else s_
    xT = np.ascontiguousarray(x.reshape(b_ * s_, e).T).astype(bf16)

    hs = [core * HPC + i for i in range(HPC)]
    perm = np.concatenate([np.arange(0, D, 2), np.arange(1, D, 2)])
    wq_c = np.concatenate(
        [w_qkv[:, 0 * e + h * D:0 * e + (h + 1) * D][:, perm] for h in hs], axis=1)
    wk_c = np.concatenate(
        [w_qkv[:, 1 * e + h * D:1 * e + (h + 1) * D][:, perm] for h in hs], axis=1)
    wv_c = np.concatenate(
        [w_qkv[:, 2 * e + h * D:2 * e + (h + 1) * D] for h in hs], axis=1)
    wp_c = np.concatenate([w_proj[h * D:(h + 1) * D, :] for h in hs], axis=0)

    inv_freq = 1.0 / (ROPE_BASE ** (np.arange(0, D, 2, dtype=np.float64) / D))
    t = np.arange(s_, dtype=np.float64)
    freqs = np.outer(inv_freq, t)            # [64, S]
    cosT = np.cos(freqs).astype(np.float32)
    sinT = np.sin(freqs).astype(np.float32)
    cos2 = np.vstack([cosT, cosT])

    sin2n = np.vstack([sinT, -sinT])  # [+sin | -sin] halves, read at the raw operand's base partition

    ki, qi = np.meshgrid(np.arange(128), np.arange(128), indexing="ij")
    trimask = (ki <= qi).astype(bf16)
    identity = np.eye(128, dtype=np.float32).astype(bf16)

    return {
        "xT": xT,
        "wq": np.ascontiguousarray(wq_c).astype(bf16),
        "wk": np.ascontiguousarray(wk_c).astype(bf16),
        "wv": np.ascontiguousarray(wv_c).astype(bf16),
        "wp": np.ascontiguousarray(wp_c).astype(np.float32),
        "cos2": cos2,
        "sin2n": sin2n.astype(np.float32), "onesf": np.ones((128, 128), np.float32),
        "trimask": trimask, "ident": identity,
        "onesd": np.ones((128, 128), np.float32).astype(bf16),
    }


_CACHE = {}


def _get_nc(b_, s_):
    key = (b_, s_)
    if key not in _CACHE:
        _CACHE[key] = build_kernel(b_, s_)
    return _CACHE[key]


def kernel(x, w_qkv, w_proj):
    x = np.asarray(x, dtype=np.float32)
    w_qkv = np.asarray(w_qkv, dtype=np.float32)
    w_proj = np.asarray(w_proj, dtype=np.float32)
    b_, s_, e = x.shape

    nc = _get_nc(b_, s_)
    in_maps = [host_inputs(x, w_qkv, w_proj, c) for c in range(N_CORES)]
    res = run_bass_kernel_spmd(nc, in_maps, list(range(N_CORES)))
    acc = res.results[0]["out"].astype(np.float32)
    for c in range(1, N_CORES):
        acc = acc + res.results[c]["out"].astype(np.float32)
    return acc.reshape(b_, s_, e)
